# revision 5
# baseline (speedup 1.0000x reference)
"""CategoryAttention (softmax over heads axis) on 8 Trainium2 cores.

Sharding: B*L = 4096 query rows split 8 ways (512 rows/core). Core c
handles batch b=c//4, query rows [(c%4)*512, (c%4+1)*512). Softmax is
over the 16 heads (local per (q,k) position) -> no cross-core comm.
Each core recomputes K/V projections for its batch (4x redundant).

v3 layout/schedule:
- All projections and attention matmuls in bf16 (FWL weight loads).
- Projections FUSED into the attention sweep via a PE filler queue:
  proj/AV matmuls are emitted between energy psum groups, so PE
  back-fills the ACT(exp) pacing gaps and HAM never re-throttles.
- Energy matmuls row-pack two heads (partitions 0-63/64-127 run as
  concurrent PE row-groups); AV matmuls col-pack the same way.
- exp runs on 4-bank psum tiles (4 heads per ACTIVATE) to amortize
  the ~0.3us per-instruction ACT overhead.
- The normalize multiply is emitted one k-tile late so the GPSIMD
  f32->bf16 reciprocal cast never stalls the DVE pipeline.
"""

import numpy as np
from collections import deque
from contextlib import ExitStack

import concourse.bass as bass
import concourse.tile as tile
from concourse import bacc, mybir
from concourse.bass_utils import run_bass_kernel_spmd

F32 = mybir.dt.float32
F32R = mybir.dt.float32r
BF16 = mybir.dt.bfloat16

N_CORES = 8
P = 128
D = 1024          # d_model
S = D // P        # 8 subtiles of the contraction dim
H = 16            # heads
HD = 64           # head dim
B = 2
L = 2048
LQ = L * B // N_CORES   # 512 query rows per core
LK = L                  # key rows per core (full batch slice)
KTS = 128               # k tile
NKT = LK // KTS         # 16
SCALE = 1.0 / np.sqrt(HD)

import os
BENCH_LOOP = int(os.environ.get("BENCH_LOOP", "1"))


def _build(has_bias):
    nc = bacc.Bacc("TRN2", target_bir_lowering=False, debug=False, num_devices=1)

    def din(name, shape, dt):
        return nc.dram_tensor(name, shape, dt, kind="ExternalInput").ap()

    qT_d = din("qT", (P, S * LQ), BF16)
    kT_d = din("kT", (P, 4 * S * 512), BF16)
    vT_d = din("vT", (P, 4 * S * 512), BF16)
    wq_d = din("wq", (P, 2 * S * 512), BF16)
    wk_d = din("wk", (P, 2 * S * 512), BF16)
    wv_d = din("wv", (P, 2 * S * 512), BF16)
    wo_d = din("wo", (P, 2 * S * 512), BF16)
    bias_d = {}
    for nm in ("bq", "bk", "bv", "bo"):
        if has_bias[nm]:
            bias_d[nm] = din(nm, (1, D), F32)
    outT_d = nc.dram_tensor("outT", (P, S * LQ), F32, kind="ExternalOutput").ap()

    qT_ap = qT_d.rearrange("p (s q) -> p s q", s=S)
    kT_ap = kT_d.rearrange("p (c s k) -> p c s k", c=4, s=S)
    vT_ap = vT_d.rearrange("p (c s k) -> p c s k", c=4, s=S)
    wq_ap = wq_d.rearrange("p (h s o) -> p h s o", h=2, s=S)
    wk_ap = wk_d.rearrange("p (h s o) -> p h s o", h=2, s=S)
    wv_ap = wv_d.rearrange("p (h s o) -> p h s o", h=2, s=S)
    wo_ap = wo_d.rearrange("p (h s o) -> p h s o", h=2, s=S)
    outT_ap = outT_d.rearrange("p (j q) -> p j q", j=S)

    with tile.TileContext(nc) as tc, ExitStack() as ctx:
        if BENCH_LOOP > 1:
            ctx.enter_context(tc.For_i(0, BENCH_LOOP, 1))

        # ---- persistent data tiles ----
        qt_pool = ctx.enter_context(tc.tile_pool(name="QT", bufs=1))
        kt_pool = ctx.enter_context(tc.tile_pool(name="KT", bufs=1))
        v_pool = ctx.enter_context(tc.tile_pool(name="V", bufs=1))
        QT_sb = qt_pool.tile([P, S, LQ], BF16)
        KT_sb = kt_pool.tile([P, S, LK], BF16)
        V_sb = v_pool.tile([P, NKT, D], BF16)

        any_bias = any(has_bias.values())
        bias_t = {}
        ones_t = None
        if any_bias:
            cpool = ctx.enter_context(tc.tile_pool(name="const", bufs=1))
            ones_t = cpool.tile([1, 512], F32, tag="ones")
            nc.vector.memset(ones_t[:], 1.0)
            for nm, d_ap in bias_d.items():
                t = cpool.tile([1, D], F32, tag=f"bias_{nm}")
                nc.sync.dma_start(t[:], d_ap)
                bias_t[nm] = t

        def bias_mm(ps_t, bias_name, o0, n_sz, o_on_partitions):
            if o_on_partitions:
                nc.tensor.matmul(ps_t, lhsT=bias_t[bias_name][0:1, o0:o0 + P],
                                 rhs=ones_t[0:1, :n_sz], start=False, stop=True)
            else:
                nc.tensor.matmul(ps_t, lhsT=ones_t[0:1, 0:P],
                                 rhs=bias_t[bias_name][0:1, o0:o0 + n_sz],
                                 start=False, stop=True)

        # psum pools (8 banks total: 2 proj + 4 energy + 2 av)
        ppsum = ctx.enter_context(tc.tile_pool(name="ppsum", bufs=1, space="PSUM"))
        e_psum = ctx.enter_context(tc.tile_pool(name="epsum", bufs=1, space="PSUM"))
        av_psum = ctx.enter_context(tc.tile_pool(name="avpsum", bufs=1, space="PSUM"))

        # ---------------- Q projection (scoped: SBUF reused later) ----
        with tc.tile_pool(name="qstream", bufs=1) as qspool, \
             tc.tile_pool(name="qwpool", bufs=2) as qwpool:
            qin = qspool.tile([P, S, LQ], BF16, tag="qin")
            nc.sync.dma_start(qin[:], qT_ap)
            wq_h = []
            for wh in range(2):
                t = qwpool.tile([P, S, 512], BF16, tag="wq")
                nc.sync.dma_start(t[:], wq_ap[:, wh])
                wq_h.append(t)
            for jp in range(4):
                ps = ppsum.tile([P, 2, 512], F32, tag="pp")
                for jj in range(2):
                    j = 2 * jp + jj
                    w_t = wq_h[j // 4]
                    jl = j % 4
                    for s in range(S):
                        nc.tensor.matmul(
                            ps[:, jj, :LQ],
                            lhsT=w_t[:, s, jl * P:(jl + 1) * P],
                            rhs=qin[:, s, :],
                            start=(s == 0),
                            stop=(s == S - 1 and not has_bias["bq"]),
                        )
                    if has_bias["bq"]:
                        bias_mm(ps[:, jj, :LQ], "bq", j * P, LQ, True)
                nc.scalar.copy(QT_sb[:, 2 * jp:2 * jp + 2, :], ps[:, :, :LQ])

        # ---- attention-era pools (allocated after Q scope frees) ----
        wk_pool = ctx.enter_context(tc.tile_pool(name="wk", bufs=2))
        kin_pool = ctx.enter_context(tc.tile_pool(name="kin", bufs=1))
        wv_pool = ctx.enter_context(tc.tile_pool(name="wv", bufs=2))
        vin_pool = ctx.enter_context(tc.tile_pool(name="vin", bufs=1))
        wo_pool = ctx.enter_context(tc.tile_pool(name="wo", bufs=1))
        attn_pool = ctx.enter_context(tc.tile_pool(name="attn", bufs=3))
        tree_pool = ctx.enter_context(tc.tile_pool(name="tree", bufs=1))
        den_pool = ctx.enter_context(tc.tile_pool(name="den", bufs=1))
        rb_pool = ctx.enter_context(tc.tile_pool(name="rb", bufs=2))
        ctx_pool = ctx.enter_context(tc.tile_pool(name="ctx", bufs=1))
        osb_pool = ctx.enter_context(tc.tile_pool(name="osb", bufs=2))

        ctx_sb = ctx_pool.tile([P, S, LQ], BF16)

        wk_h = []
        for wh in range(2):
            t = wk_pool.tile([P, S, 512], BF16, tag="wk")
            nc.sync.dma_start(t[:], wk_ap[:, wh])
            wk_h.append(t)
        wv_h = []
        for wh in range(2):
            t = wv_pool.tile([P, S, 512], BF16, tag="wv")
            nc.sync.dma_start(t[:], wv_ap[:, wh])
            wv_h.append(t)

        kin_cur = [None]
        vin_cur = [None]

        def dma_kin(kn):
            t = kin_pool.tile([P, S, 512], BF16, tag="kin")
            nc.sync.dma_start(t[:], kT_ap[:, kn])
            kin_cur[0] = t

        def dma_vin(kn):
            t = vin_pool.tile([P, S, 512], BF16, tag="vin")
            nc.sync.dma_start(t[:], vT_ap[:, kn])
            vin_cur[0] = t

        def k_chunk_jpair(kn, jp):
            """Project K cols [kn*512,+512) for o-tiles 2jp, 2jp+1."""
            kin = kin_cur[0]
            ps = ppsum.tile([P, 2, 512], F32, tag="pp")
            for jj in range(2):
                j = 2 * jp + jj
                w_t = wk_h[j // 4]
                jl = j % 4
                for s in range(S):
                    nc.tensor.matmul(
                        ps[:, jj, :],
                        lhsT=w_t[:, s, jl * P:(jl + 1) * P],
                        rhs=kin[:, s, :],
                        start=(s == 0),
                        stop=(s == S - 1 and not has_bias["bk"]),
                    )
                if has_bias["bk"]:
                    bias_mm(ps[:, jj, :], "bk", j * P, 512, True)
            nc.scalar.copy(
                KT_sb[:, 2 * jp:2 * jp + 2, kn * 512:(kn + 1) * 512], ps[:])

        def v_chunk_kt4(kn, kt4):
            """Project V rows for k-tile kn*4+kt4 (128 rows, 1024 cols)."""
            vin = vin_cur[0]
            kt = kn * 4 + kt4
            ps = ppsum.tile([P, 2, 512], F32, tag="pp")
            for t in range(2):
                for s in range(S):
                    nc.tensor.matmul(
                        ps[:, t, :],
                        lhsT=vin[:, s, kt4 * P:(kt4 + 1) * P],
                        rhs=wv_h[t][:, s, :],
                        start=(s == 0),
                        stop=(s == S - 1 and not has_bias["bv"]),
                    )
                if has_bias["bv"]:
                    bias_mm(ps[:, t, :], "bv", t * 512, 512, False)
            nc.scalar.copy(V_sb[:, kt, :],
                           ps[:].rearrange("p a b -> p (a b)"))

        # ---------------- attention ----------------
        filler_q = deque()

        def pump(n=1):
            for _ in range(n):
                if filler_q:
                    filler_q.popleft()()

        def softmax_s1(kt):
            """Energy (row-packed pairs) -> exp (4 heads/ACTIVATE) -> den
            -> reciprocal (bf16 via gpsimd). Normalize mul is deferred."""
            attn_t = attn_pool.tile([P, H, LQ], BF16, tag="attn")
            for g4 in range(4):
                eps = e_psum.tile([P, 4, LQ], F32, tag="e")
                for hh in range(4):
                    h = 4 * g4 + hh
                    j2, p0 = h // 2, HD * (h % 2)
                    nc.tensor.matmul(
                        eps[:, hh, :],
                        lhsT=KT_sb[p0:p0 + HD, j2, kt * KTS:(kt + 1) * KTS],
                        rhs=QT_sb[p0:p0 + HD, j2, :],
                        start=True,
                        stop=True,
                    )
                nc.scalar.activation(attn_t[:, 4 * g4:4 * (g4 + 1), :], eps[:],
                                     mybir.ActivationFunctionType.Exp,
                                     scale=float(SCALE))
                pump(1)
            # den = sum over heads (bf16 tree at DVE 2x; final add f32)
            t1 = tree_pool.tile([P, 4, LQ], BF16)
            with nc.allow_low_precision(reason="bf16 head-sum tree"):
                nc.vector.tensor_add(t1[:], attn_t[:, 0:4, :], attn_t[:, 4:8, :])
                nc.vector.tensor_add(t1[:], t1[:], attn_t[:, 8:12, :])
                nc.vector.tensor_add(t1[:], t1[:], attn_t[:, 12:16, :])
                nc.vector.tensor_add(t1[:, 0:2, :], t1[:, 0:2, :], t1[:, 2:4, :])
            den = den_pool.tile([P, 2, LQ], F32)
            nc.vector.tensor_add(den[:, 0, :], t1[:, 0, :], t1[:, 1, :])
            nc.vector.reciprocal_approx_fast(den[:, 1, :], den[:, 0, :])
            rb = rb_pool.tile([P, LQ], BF16, tag="rb")
            nc.gpsimd.tensor_copy(rb[:], den[:, 1, :])
            return attn_t, rb

        def softmax_s2(attn_t, rb):
            nc.vector.tensor_mul(
                attn_t[:], attn_t[:],
                rb[:, None, :].to_broadcast((P, H, LQ)))

        def av_group(u, c0, attn_list, first):
            """One avp tile: heads 4u..4u+3, full q, over 2 k-tiles."""
            avp = av_psum.tile([P, 2, LQ], F32, tag="av")
            for ci in range(2):
                kt = c0 + ci
                for hh in range(4):
                    h = 4 * u + hh
                    i, p0 = hh // 2, HD * (hh % 2)
                    nc.tensor.matmul(
                        avp[p0:p0 + HD, i, :],
                        lhsT=V_sb[:, kt, h * HD:(h + 1) * HD],
                        rhs=attn_list[ci][:, h, :],
                        start=(ci == 0),
                        stop=(ci == 1),
                    )
            with nc.allow_low_precision(reason="bf16 ctx accumulate"):
                if first:
                    nc.vector.tensor_copy(ctx_sb[:, 2 * u:2 * u + 2, :],
                                          avp[:, :, :])
                else:
                    nc.vector.tensor_add(ctx_sb[:, 2 * u:2 * u + 2, :],
                                         ctx_sb[:, 2 * u:2 * u + 2, :],
                                         avp[:, :, :])

        # per-pair filler schedule: projection chunks + input DMAs.
        # K-chunk kn must be fully emitted before pair 2*kn (its k-tiles);
        # V-chunk kn quarters (k-tiles) before the av groups that read them
        # (av for pair p runs during pair p+1).
        def pair_fillers(p):
            K = k_chunk_jpair
            V = v_chunk_kt4
            fs = {
                0: [lambda: dma_kin(1),
                    lambda: V(0, 0), lambda: V(0, 1)],
                1: [lambda: V(0, 2), lambda: V(0, 3),
                    lambda: K(1, 0), lambda: K(1, 1), lambda: K(1, 2),
                    lambda: K(1, 3), lambda: dma_vin(1)],
                2: [lambda: V(1, 0), lambda: V(1, 1), lambda: dma_kin(2)],
                3: [lambda: V(1, 2), lambda: V(1, 3),
                    lambda: K(2, 0), lambda: K(2, 1), lambda: K(2, 2),
                    lambda: K(2, 3), lambda: dma_vin(2)],
                4: [lambda: V(2, 0), lambda: V(2, 1), lambda: dma_kin(3)],
                5: [lambda: V(2, 2), lambda: V(2, 3),
                    lambda: K(3, 0), lambda: K(3, 1), lambda: K(3, 2),
                    lambda: K(3, 3), lambda: dma_vin(3)],
                6: [lambda: V(3, 0), lambda: V(3, 1), dma_wo0],
                7: [lambda: V(3, 2), lambda: V(3, 3)],
            }
            return fs[p]

        wo_tiles = []

        def dma_wo0():
            t = wo_pool.tile([P, S, 512], BF16, tag="wo")
            nc.sync.dma_start(t[:], wo_ap[:, 0])
            wo_tiles.append(t)

        # prologue: K chunk 0 (all 8 o-tiles)
        dma_kin(0)
        dma_vin(0)
        for jp in range(4):
            k_chunk_jpair(0, jp)

        prev = None  # (c0, [attn_kt0, attn_kt1])
        for p in range(8):
            c0 = 2 * p
            # av groups FIRST: they must be emitted before softmax_s1(c0+1)
            # allocates an attn ring slot whose previous occupant they read.
            if prev is not None:
                for u in range(4):
                    filler_q.append(
                        lambda u=u, pr=prev: av_group(u, pr[0], pr[1],
                                                      pr[0] == 0))
            for f in pair_fillers(p):
                filler_q.append(f)
            a0, r0 = softmax_s1(c0)
            a1, r1 = softmax_s1(c0 + 1)
            pump(len(filler_q))
            softmax_s2(a0, r0)
            softmax_s2(a1, r1)
            prev = (c0, [a0, a1])
        for u in range(4):
            av_group(u, prev[0], prev[1], False)

        # ---------------- output projection ----------------
        for j4 in range(2):
            if j4 == 0 and wo_tiles:
                woh = wo_tiles[0]
            else:
                woh = wo_pool.tile([P, S, 512], BF16, tag="wo")
                nc.sync.dma_start(woh[:], wo_ap[:, j4])
            for j2 in range(2):
                po = av_psum.tile([P, 2, LQ], F32, tag="av")
                for jj in range(2):
                    j = j4 * 4 + j2 * 2 + jj
                    jl = j2 * 2 + jj
                    for s in range(S):
                        nc.tensor.matmul(
                            po[:, jj, :],
                            lhsT=woh[:, s, jl * P:(jl + 1) * P],
                            rhs=ctx_sb[:, s, :],
                            start=(s == 0),
                            stop=(s == S - 1 and not has_bias["bo"]),
                        )
                    if has_bias["bo"]:
                        bias_mm(po[:, jj, :], "bo", j * P, LQ, True)
                osb = osb_pool.tile([P, 2, LQ], F32, tag="osb")
                nc.scalar.copy(osb[:], po[:])
                j0 = j4 * 4 + j2 * 2
                nc.sync.dma_start(outT_ap[:, j0:j0 + 2, :], osb[:])

    nc.compile()
    return nc


_cache = {}


def _get_program(has_bias):
    key = (BENCH_LOOP, tuple(sorted(has_bias.items())))
    if key not in _cache:
        _cache[key] = _build(has_bias)
    return _cache[key]


def _part_major(x):
    n = x.shape[1]
    return np.ascontiguousarray(
        x.reshape(S, P, n).transpose(1, 0, 2).reshape(P, S * n))


def _chunked(x, width=512):
    """[D, N] -> [P, N//width, S, width] per-chunk contiguous layout."""
    n = x.shape[1]
    nch = n // width
    y = x.reshape(S, P, nch, width).transpose(1, 2, 0, 3)
    return np.ascontiguousarray(y.reshape(P, nch * S * width))


def _bf16(x):
    import ml_dtypes
    return np.ascontiguousarray(x).astype(ml_dtypes.bfloat16)


def prepare_inputs(query, key, value, Wq_w, Wq_b, Wk_w, Wk_b, Wv_w, Wv_b,
                   Wo_w, Wo_b):
    query = np.asarray(query, dtype=np.float32)
    key = np.asarray(key, dtype=np.float32)
    value = np.asarray(value, dtype=np.float32)
    w = {
        "wq": _bf16(_chunked(np.ascontiguousarray(np.asarray(Wq_w, np.float32).T))),
        "wk": _bf16(_chunked(np.ascontiguousarray(np.asarray(Wk_w, np.float32).T))),
        "wv": _bf16(_chunked(np.ascontiguousarray(np.asarray(Wv_w, np.float32).T))),
        "wo": _bf16(_chunked(np.ascontiguousarray(np.asarray(Wo_w, np.float32).T))),
    }
    biases = {"bq": np.asarray(Wq_b, np.float32), "bk": np.asarray(Wk_b, np.float32),
              "bv": np.asarray(Wv_b, np.float32), "bo": np.asarray(Wo_b, np.float32)}
    has_bias = {nm: bool(np.any(b)) for nm, b in biases.items()}

    kT = [_bf16(_chunked(np.ascontiguousarray(key[b].T))) for b in range(B)]
    vT = [_bf16(_chunked(np.ascontiguousarray(value[b].T))) for b in range(B)]

    in_maps = []
    for c in range(N_CORES):
        b, qc = c // (N_CORES // B), c % (N_CORES // B)
        qslice = query[b, qc * LQ:(qc + 1) * LQ, :]
        m = {
            "qT": _bf16(_part_major(np.ascontiguousarray(qslice.T))),
            "kT": kT[b],
            "vT": vT[b],
            **w,
        }
        for nm, hb in has_bias.items():
            if hb:
                m[nm] = biases[nm].reshape(1, D)
        in_maps.append(m)
    return in_maps, has_bias


def gather_output(results):
    out = np.empty((B, L, D), dtype=np.float32)
    for c in range(N_CORES):
        b, qc = c // (N_CORES // B), c % (N_CORES // B)
        oT = results[c]["outT"].reshape(P, S, LQ).transpose(1, 0, 2).reshape(D, LQ)
        out[b, qc * LQ:(qc + 1) * LQ, :] = oT.T
    return out


def kernel(**inputs) -> np.ndarray:
    in_maps, has_bias = prepare_inputs(**inputs)
    nc = _get_program(has_bias)
    res = run_bass_kernel_spmd(nc, in_maps, list(range(N_CORES)))
    return gather_output(res.results)
